# revision 39
# baseline (speedup 1.0000x reference)
"""Trainium2 Bass kernel for nn_ODEFunc_90159953478502 (MoE routing, inference path).

Math (see reference):
    logits  = x @ Wg[:256] + (t*Wg[512] + bg)      # zeros kill Wg[256:512]
    w       = softmax(logits, axis=-1)             # [B, E]
    eo_e    = tanh(x @ W1[e] + b1[e]) @ W2[e] + b2[e]
    active_e = any_b(w[b,e] > 0.01)                # always >=1 active:
    out     = sum_e active_e * w[:,e,None] * eo_e  # max softmax >= 1/8 > 0.01

Sharding: expert-parallel. Core e holds the full batch plus only W1[e]/W2[e],
computes w[:,e,None] * (tanh(x@W1[e]+b1[e]) @ W2[e]) in transposed layout
([D, B]); the host applies the 0/1 active mask (exported as MACT) and sums
the 8 partial outputs. The b2 rank-1 term (zero for this problem) would be
added host-side from a numpy gating pass.

Device structure per core (all bf16 matmuls, fp32 accumulation):
  - x arrives pre-transposed (xT [D, B]) so W1/W2/Wg act as matmul lhsT in
    natural layout (out = lhsT.T @ rhs, contraction on partitions).
  - gating runs in [E, B] layout: logits^T [8, B] chunks via lhsT=Wg_x,
    ACT Exp with fused +gbias (no max-subtract: |logits| <= ~6). Wg's
    columns are PERMUTED per core so the core's own expert is partition 0:
    E_e is a plain [0:1] slice; only the ones-column sum S needs a matmul.
  - w_e = E_e * reciprocal_approx_fast(S); the [128, B]-broadcast wb is a
    PE outer product ones[1,128]^T @ w_e (PSUM) + DVE copy — no DRAM
    bounce, keeping the software-DMA queue off the critical path.
  - active mask = (max_b w_e) > 0.01, reduced per chunk on DVE and
    exported as MACT [1,1]; applied host-side.
  - mm1(c+1) and mm2(c) interleave per h-tile on the PE so the ACT engine
    (684 ns/tanh tile vs PE's ~426 ns/psh tile) drains PSUM continuously
    instead of backing up the psh ring.
  - per-drain weighting is one DVE multiply: out^T tile = pso * wb.
"""

import sys

if "/opt/trn_rl_repo" not in sys.path:
    sys.path.insert(0, "/opt/trn_rl_repo")

import numpy as np

_B, _D, _H, _E = 4096, 256, 1024, 8
_NCORES = 8
_CHUNK = 512
_NCH = _B // _CHUNK
_DT = _D // 128   # 2 d-tiles
_HT = _H // 128   # 8 h-tiles
_THRESH = 0.01

_MM_BF16 = True  # main-matmul dtype: False -> float32r, True -> bfloat16

_CACHE = {}


def _build(mm_bf16):
    import concourse.bass as bass
    import concourse.tile as tile
    import concourse.mybir as mybir
    from concourse import bacc
    from contextlib import ExitStack

    F32 = mybir.dt.float32
    F32R = mybir.dt.float32r
    BF16 = mybir.dt.bfloat16
    MMDT = BF16 if mm_bf16 else F32R
    MMIO = BF16 if mm_bf16 else F32  # dram dtype of x/W1/W2/Wg_x
    AF = mybir.ActivationFunctionType
    ALU = mybir.AluOpType
    AX = mybir.AxisListType

    nc = bacc.Bacc("TRN2", target_bir_lowering=False, debug=False)

    XT = nc.declare_dram_parameter("XT", [_D, _B], MMIO, isOutput=False)
    W1E = nc.declare_dram_parameter("W1E", [_D, _H], MMIO, isOutput=False)
    W2E = nc.declare_dram_parameter("W2E", [_H, _D], MMIO, isOutput=False)
    B1E = nc.declare_dram_parameter("B1E", [128, _HT], F32, isOutput=False)
    WGX = nc.declare_dram_parameter("WGX", [_D, _E], MMIO, isOutput=False)
    GB = nc.declare_dram_parameter("GB", [_E, 1], F32, isOutput=False)
    ONESM = nc.declare_dram_parameter("ONESM", [_E, 128], F32, isOutput=False)
    OUTT = nc.declare_dram_parameter("OUTT", [_D, _B], MMIO, isOutput=True)
    MACT = nc.declare_dram_parameter("MACT", [1, 1], F32, isOutput=True)

    with tile.TileContext(nc) as tc, ExitStack() as ctx:
        const = ctx.enter_context(tc.tile_pool(name="const", bufs=1))
        epool = ctx.enter_context(tc.tile_pool(name="epool", bufs=4))
        small = ctx.enter_context(tc.tile_pool(name="small", bufs=4))
        wrp = ctx.enter_context(tc.tile_pool(name="wrp", bufs=4))
        wbp = ctx.enter_context(tc.tile_pool(name="wbp", bufs=3))
        crp = ctx.enter_context(tc.tile_pool(name="crp", bufs=12))
        htp = ctx.enter_context(tc.tile_pool(name="htp", bufs=18))
        op = ctx.enter_context(tc.tile_pool(name="op", bufs=4))
        pgs = ctx.enter_context(tc.tile_pool(name="pgs", bufs=2, space="PSUM"))
        ph = ctx.enter_context(tc.tile_pool(name="ph", bufs=4, space="PSUM"))
        po = ctx.enter_context(tc.tile_pool(name="po", bufs=2, space="PSUM"))

        # ---- inputs: consolidated DMA triggers (each costs ~650ns serial
        # time on the Sync queue), plain 2-D slices only, critical-path order
        def xt_slice(d, c0, c1):
            return XT.ap()[
                d * 128 : (d + 1) * 128, c0 * _CHUNK : c1 * _CHUNK
            ].bitcast(MMDT)

        # Sync queue: main-path loads. GpSimd queue (otherwise idle): the
        # rest — trigger serialization runs on both queues in parallel.
        # x chunk 0 first: gating(0) needs it
        x0d = []
        for d in range(_DT):
            t = const.tile([128, _CHUNK], MMDT, tag=f"x0_{d}")
            nc.sync.dma_start(t[:], xt_slice(d, 0, 1))
            x0d.append(t)
        gb_sb = const.tile([_E, 1], F32)
        nc.gpsimd.dma_start(gb_sb[:], GB.ap())
        onesm = const.tile([_E, 128], F32R)
        nc.gpsimd.dma_start(onesm[:], ONESM.ap().bitcast(F32R))
        b1_sb = const.tile([128, _HT], F32)
        nc.gpsimd.dma_start(b1_sb[:], B1E.ap())
        # W1 rows: one trigger per d-block
        w1d = []
        for d in range(_DT):
            t = const.tile([128, _H], MMDT, tag=f"w1_{d}")
            nc.sync.dma_start(
                t[:], W1E.ap()[d * 128 : (d + 1) * 128, :].bitcast(MMDT)
            )
            w1d.append(t)
        wgx_sb = const.tile([128, _DT * _E], MMDT)
        for d in range(_DT):
            nc.sync.dma_start(
                wgx_sb[:, d * _E : (d + 1) * _E],
                WGX.ap()[d * 128 : (d + 1) * 128, :].bitcast(MMDT),
            )
        # warmup activation: forces the ACT table-load to the front of the
        # Scalar queue so the first real Exp isn't gated on input DMA.
        warm = const.tile([1, 2], MMDT)
        nc.scalar.activation(warm[:], wgx_sb[0:1, 0:2], AF.Tanh)
        # W2: one trigger per hh-block, on the gpsimd queue
        w2d = []
        for hh in range(_HT):
            t = const.tile([128, _D], MMDT, tag=f"w2_{hh}")
            nc.gpsimd.dma_start(
                t[:], W2E.ap()[hh * 128 : (hh + 1) * 128, :].bitcast(MMDT)
            )
            w2d.append(t)
        # x chunk 1
        x1d = []
        for d in range(_DT):
            t = const.tile([128, _CHUNK], MMDT, tag=f"x1_{d}")
            nc.sync.dma_start(t[:], xt_slice(d, 1, 2))
            x1d.append(t)
        # x chunks 2..7: one big trigger per d-block
        _XR = (_NCH - 2) * _CHUNK
        xrd = []
        for d in range(_DT):
            t = const.tile([128, _XR], MMDT, tag=f"xr_{d}")
            nc.gpsimd.dma_start(t[:], xt_slice(d, 2, _NCH))
            xrd.append(t)

        def xm(d, c):
            if c == 0:
                return x0d[d][:]
            if c == 1:
                return x1d[d][:]
            o = (c - 2) * _CHUNK
            return xrd[d][:, o : o + _CHUNK]

        def w1v(d, hh):
            return w1d[d][:, hh * 128 : (hh + 1) * 128]

        def w2v(hh, d2):
            return w2d[hh][:, d2 * 128 : (d2 + 1) * 128]

        # ---- main loop: one iteration computes gating+mm1 of chunk n while
        # running mm2+drain of chunk c, interleaved per h-tile on the PE so
        # gating's ACT/DVE round-trips hide behind matmul work ---------------
        wb_tiles = {}
        e_by_chunk = {}
        ht_by_chunk = {}
        macc = const.tile([1, 1], F32)

        def gating_head(n):
            # logits -> exp; PE + ACT only, no downstream waits
            psg = pgs.tile([_E, _CHUNK], F32, tag="pg")
            for d in range(_DT):
                nc.tensor.matmul(
                    psg[:], wgx_sb[:, d * _E : (d + 1) * _E], xm(d, n),
                    start=(d == 0), stop=(d == _DT - 1),
                )
            e_sb = epool.tile([_E, _CHUNK], F32R, tag="e_sb")
            nc.scalar.activation(e_sb[:], psg[:], AF.Exp, bias=gb_sb[:])
            e_by_chunk[n] = e_sb

        def gating_sum(n):
            # S = column sum over experts; own expert's row is partition 0
            e_sb = e_by_chunk[n]
            pss = pgs.tile([1, _CHUNK], F32, tag="pg")
            nc.tensor.matmul(
                pss[:], onesm[:, 0:1], e_sb[:], start=True, stop=True
            )
            recip = small.tile([1, _CHUNK], F32, tag="recip")
            nc.vector.reciprocal_approx_fast(recip[:], pss[0:1, :])
            wu = wrp.tile([1, _CHUNK], F32R, tag="wu")
            nc.vector.tensor_tensor(
                wu[:], e_sb[0:1, :].bitcast(F32), recip[:], ALU.mult
            )
            return wu

        def gating_wb(n, wu):
            # wb[128, CHUNK] = ones[1,128].T @ wu — PE outer-product bcast
            wbps = pgs.tile([128, _CHUNK], F32, tag="pg")
            nc.tensor.matmul(
                wbps[:], onesm[0:1, :], wu[:], start=True, stop=True
            )
            wb = wbp.tile([128, _CHUNK], F32, tag="wb")
            nc.vector.tensor_copy(wb[:], wbps[:])
            wb_tiles[n] = wb
            del e_by_chunk[n]
            # active criterion: max_b w > thresh, max-accumulated into macc
            rmax = crp.tile([1, 1], F32, tag="rmax", name=f"rmax_{n}")
            nc.vector.reduce_max(rmax[:], wu[:].bitcast(F32), axis=AX.X)
            if n == 0:
                nc.vector.tensor_copy(macc[:], rmax[:])
            else:
                nc.vector.tensor_tensor(macc[:], macc[:], rmax[:], ALU.max)

        def mm1_tile(n, hh):
            psh = ph.tile([128, _CHUNK], F32, tag="psh")
            for d in range(_DT):
                nc.tensor.matmul(
                    psh[:], w1v(d, hh), xm(d, n),
                    start=(d == 0), stop=(d == _DT - 1),
                )
            ht = htp.tile([128, _CHUNK], MMDT, tag="ht")
            nc.scalar.activation(ht[:], psh[:], AF.Tanh, bias=b1_sb[:, hh : hh + 1])
            ht_by_chunk.setdefault(n, []).append(ht)

        def iteration(n, c):
            # n: chunk for gating+mm1 (None to skip); c: chunk for mm2+drain
            pso_tiles = None
            ht_prev = None
            if c is not None:
                pso_tiles = [
                    po.tile([128, _CHUNK], F32, tag="pso", name=f"pso_{c}_{d2}")
                    for d2 in range(_DT)
                ]
                ht_prev = ht_by_chunk.pop(c)
            if n is not None:
                gating_head(n)
                wu = None
            for hh in range(_HT):
                if n is not None:
                    mm1_tile(n, hh)
                if c is not None:
                    for d2 in range(_DT):
                        nc.tensor.matmul(
                            pso_tiles[d2][:],
                            w2v(hh, d2),
                            ht_prev[hh][:],
                            start=(hh == 0), stop=(hh == _HT - 1),
                        )
                if n is not None and hh == 0:
                    wu = gating_sum(n)
                if n is not None and hh == 2:
                    gating_wb(n, wu)
            if c is not None:
                cs = slice(c * _CHUNK, (c + 1) * _CHUNK)
                wb = wb_tiles.pop(c)
                for d2 in range(_DT):
                    osb = op.tile([128, _CHUNK], MMIO, tag="osb")
                    nc.vector.tensor_tensor(
                        osb[:], pso_tiles[d2][:], wb[:], ALU.mult
                    )
                    q = nc.sync if d2 == 0 else nc.gpsimd
                    q.dma_start(
                        OUTT.ap()[d2 * 128 : (d2 + 1) * 128, cs], osb[:]
                    )

        iteration(0, None)
        for c in range(_NCH):
            n = c + 1
            iteration(n if n < _NCH else None, c)

        # active mask -> MACT [1,1] (exact 0.0/1.0), applied host-side
        mact = crp.tile([1, 1], F32, tag="mact")
        nc.vector.tensor_scalar(mact[:], macc[:], _THRESH, None, ALU.is_gt)
        nc.sync.dma_start(MACT.ap(), mact[:])

    nc.finalize()
    return nc


def _get_nc():
    key = ("nc", _MM_BF16)
    if key not in _CACHE:
        _CACHE[key] = _build(_MM_BF16)
    return _CACHE[key]


def _make_in_maps(t, x, W1, b1, W2, b2, Wg, bg):
    import ml_dtypes

    mmdt = ml_dtypes.bfloat16 if _MM_BF16 else np.float32
    xTm = np.ascontiguousarray(x.T.astype(mmdt))
    wgx = Wg[:_D].astype(mmdt)
    gb = (np.float32(t[0]) * Wg[2 * _D] + bg).astype(np.float32).reshape(_E, 1)
    onesm = np.ones((_E, 128), dtype=np.float32)
    in_maps = []
    for c in range(_NCORES):
        # own expert first: E_e lands on partition 0 of the gating layout
        perm = [c] + [e for e in range(_E) if e != c]
        in_maps.append(
            {
                "XT": xTm,
                "W1E": np.ascontiguousarray(W1[c].astype(mmdt)),
                "W2E": np.ascontiguousarray(W2[c].astype(mmdt)),
                "B1E": np.ascontiguousarray(
                    b1[c].reshape(_HT, 128).T, dtype=np.float32
                ),
                "WGX": np.ascontiguousarray(wgx[:, perm]),
                "GB": np.ascontiguousarray(gb[perm]),
                "ONESM": onesm,
            }
        )
    return in_maps


def _assemble(results, inputs):
    out = np.zeros((_B, _D), dtype=np.float64)
    for c in range(_NCORES):
        if results[c]["MACT"][0, 0] > 0.5:
            out += results[c]["OUTT"].astype(np.float64).T
    b2 = np.asarray(inputs["b2"])
    if np.any(b2):
        # rank-1 bias term sum_e m_e * w[:,e] b2[e,:] — numpy gating replay
        t, x, Wg, bg = (np.asarray(inputs[k]) for k in ("t", "x", "Wg", "bg"))
        logits = x.astype(np.float64) @ Wg[:_D].astype(np.float64)
        logits += np.float64(t[0]) * Wg[2 * _D].astype(np.float64) + bg
        ex = np.exp(logits - logits.max(axis=1, keepdims=True))
        w = ex / ex.sum(axis=1, keepdims=True)
        active = (w > _THRESH).any(axis=0)
        out += (w * active) @ b2.astype(np.float64)
    return out.astype(np.float32)


def run_on_device(t, x, W1, b1, W2, b2, Wg, bg, trace=False):
    from concourse.bass_utils import run_bass_kernel_spmd

    inputs = dict(t=t, x=x, W1=W1, b1=b1, W2=W2, b2=b2, Wg=Wg, bg=bg)
    in_maps = _make_in_maps(**inputs)
    res = run_bass_kernel_spmd(
        _get_nc(), in_maps, list(range(_NCORES)), trace=trace
    )
    return _assemble(res.results, inputs), res


def kernel(t, x, W1, b1, W2, b2, Wg, bg):
    out, _ = run_on_device(t, x, W1, b1, W2, b2, Wg, bg, trace=False)
    return out


# revision 41
# speedup vs baseline: 1.0191x; 1.0191x over previous
"""Trainium2 Bass kernel for nn_ODEFunc_90159953478502 (MoE routing, inference path).

Math (see reference):
    logits  = x @ Wg[:256] + (t*Wg[512] + bg)      # zeros kill Wg[256:512]
    w       = softmax(logits, axis=-1)             # [B, E]
    eo_e    = tanh(x @ W1[e] + b1[e]) @ W2[e] + b2[e]
    active_e = any_b(w[b,e] > 0.01)                # always >=1 active:
    out     = sum_e active_e * w[:,e,None] * eo_e  # max softmax >= 1/8 > 0.01

Sharding: expert-parallel. Core e holds the full batch plus only W1[e]/W2[e],
computes w[:,e,None] * (tanh(x@W1[e]+b1[e]) @ W2[e]) in transposed layout
([D, B]); the host applies the 0/1 active mask (exported as MACT) and sums
the 8 partial outputs. The b2 rank-1 term (zero for this problem) would be
added host-side from a numpy gating pass.

Device structure per core (all bf16 matmuls, fp32 accumulation):
  - x arrives pre-transposed (xT [D, B]) so W1/W2/Wg act as matmul lhsT in
    natural layout (out = lhsT.T @ rhs, contraction on partitions).
  - gating runs in [E, B] layout: logits^T [8, B] chunks via lhsT=Wg_x,
    ACT Exp with fused +gbias (no max-subtract: |logits| <= ~6). Wg's
    columns are PERMUTED per core so the core's own expert is partition 0:
    E_e is a plain [0:1] slice; only the ones-column sum S needs a matmul.
  - w_e = E_e * reciprocal_approx_fast(S); the [128, B]-broadcast wb is a
    PE outer product ones[1,128]^T @ w_e (PSUM) + DVE copy — no DRAM
    bounce, keeping the software-DMA queue off the critical path.
  - active mask = (max_b w_e) > 0.01, reduced per chunk on DVE and
    exported as MACT [1,1]; applied host-side.
  - mm1(c+1) and mm2(c) interleave per h-tile on the PE so the ACT engine
    (684 ns/tanh tile vs PE's ~426 ns/psh tile) drains PSUM continuously
    instead of backing up the psh ring.
  - per-drain weighting is one DVE multiply: out^T tile = pso * wb.
"""

import sys

if "/opt/trn_rl_repo" not in sys.path:
    sys.path.insert(0, "/opt/trn_rl_repo")

import numpy as np

_B, _D, _H, _E = 4096, 256, 1024, 8
_NCORES = 8
_CHUNK = 512
_NCH = _B // _CHUNK
_DT = _D // 128   # 2 d-tiles
_HT = _H // 128   # 8 h-tiles
_THRESH = 0.01

_MM_BF16 = True  # main-matmul dtype: False -> float32r, True -> bfloat16

_CACHE = {}


def _build(mm_bf16):
    import concourse.bass as bass
    import concourse.tile as tile
    import concourse.mybir as mybir
    from concourse import bacc
    from contextlib import ExitStack

    F32 = mybir.dt.float32
    F32R = mybir.dt.float32r
    BF16 = mybir.dt.bfloat16
    MMDT = BF16 if mm_bf16 else F32R
    MMIO = BF16 if mm_bf16 else F32  # dram dtype of x/W1/W2/Wg_x
    AF = mybir.ActivationFunctionType
    ALU = mybir.AluOpType
    AX = mybir.AxisListType

    nc = bacc.Bacc("TRN2", target_bir_lowering=False, debug=False)

    XT = nc.declare_dram_parameter("XT", [_D, _B], MMIO, isOutput=False)
    W1E = nc.declare_dram_parameter("W1E", [_D, _H], MMIO, isOutput=False)
    W2E = nc.declare_dram_parameter("W2E", [_H, _D], MMIO, isOutput=False)
    B1E = nc.declare_dram_parameter("B1E", [128, _HT], F32, isOutput=False)
    WGX = nc.declare_dram_parameter("WGX", [_D, _E], MMIO, isOutput=False)
    GB = nc.declare_dram_parameter("GB", [_E, 1], F32, isOutput=False)
    ONESM = nc.declare_dram_parameter("ONESM", [_E, 128], F32, isOutput=False)
    OUTT = nc.declare_dram_parameter("OUTT", [_D, _B], MMIO, isOutput=True)
    MACT = nc.declare_dram_parameter("MACT", [1, 1], F32, isOutput=True)

    with tile.TileContext(nc) as tc, ExitStack() as ctx:
        const = ctx.enter_context(tc.tile_pool(name="const", bufs=1))
        epool = ctx.enter_context(tc.tile_pool(name="epool", bufs=4))
        small = ctx.enter_context(tc.tile_pool(name="small", bufs=4))
        wrp = ctx.enter_context(tc.tile_pool(name="wrp", bufs=4))
        wbp = ctx.enter_context(tc.tile_pool(name="wbp", bufs=3))
        crp = ctx.enter_context(tc.tile_pool(name="crp", bufs=12))
        htp = ctx.enter_context(tc.tile_pool(name="htp", bufs=18))
        op = ctx.enter_context(tc.tile_pool(name="op", bufs=4))
        pgs = ctx.enter_context(tc.tile_pool(name="pgs", bufs=2, space="PSUM"))
        ph = ctx.enter_context(tc.tile_pool(name="ph", bufs=4, space="PSUM"))
        po = ctx.enter_context(tc.tile_pool(name="po", bufs=2, space="PSUM"))

        # ---- inputs: consolidated DMA triggers (each costs ~650ns serial
        # time on the Sync queue), plain 2-D slices only, critical-path order
        def xt_slice(d, c0, c1):
            return XT.ap()[
                d * 128 : (d + 1) * 128, c0 * _CHUNK : c1 * _CHUNK
            ].bitcast(MMDT)

        # x chunk 0 first: gating(0) needs it
        x0d = []
        for d in range(_DT):
            t = const.tile([128, _CHUNK], MMDT, tag=f"x0_{d}")
            nc.sync.dma_start(t[:], xt_slice(d, 0, 1))
            x0d.append(t)
        wgx_sb = const.tile([128, _DT * _E], MMDT)
        for d in range(_DT):
            nc.sync.dma_start(
                wgx_sb[:, d * _E : (d + 1) * _E],
                WGX.ap()[d * 128 : (d + 1) * 128, :].bitcast(MMDT),
            )
        # warmup activation: forces the ACT table-load to the front of the
        # Scalar queue so the first real Exp isn't gated on input DMA.
        warm = const.tile([1, 2], MMDT)
        nc.scalar.activation(warm[:], wgx_sb[0:1, 0:2], AF.Tanh)
        # W1 rows: one trigger per d-block
        w1d = []
        for d in range(_DT):
            t = const.tile([128, _H], MMDT, tag=f"w1_{d}")
            nc.sync.dma_start(
                t[:], W1E.ap()[d * 128 : (d + 1) * 128, :].bitcast(MMDT)
            )
            w1d.append(t)
        gb_sb = const.tile([_E, 1], F32)
        nc.sync.dma_start(gb_sb[:], GB.ap())
        onesm = const.tile([_E, 128], F32R)
        nc.sync.dma_start(onesm[:], ONESM.ap().bitcast(F32R))
        b1_sb = const.tile([128, _HT], F32)
        nc.sync.dma_start(b1_sb[:], B1E.ap())
        # x chunk 1
        x1d = []
        for d in range(_DT):
            t = const.tile([128, _CHUNK], MMDT, tag=f"x1_{d}")
            nc.sync.dma_start(t[:], xt_slice(d, 1, 2))
            x1d.append(t)
        # W2: one trigger per hh-block
        w2d = []
        for hh in range(_HT):
            t = const.tile([128, _D], MMDT, tag=f"w2_{hh}")
            nc.sync.dma_start(
                t[:], W2E.ap()[hh * 128 : (hh + 1) * 128, :].bitcast(MMDT)
            )
            w2d.append(t)
        # x chunks 2..7: one big trigger per d-block
        _XR = (_NCH - 2) * _CHUNK
        xrd = []
        for d in range(_DT):
            t = const.tile([128, _XR], MMDT, tag=f"xr_{d}")
            nc.sync.dma_start(t[:], xt_slice(d, 2, _NCH))
            xrd.append(t)

        def xm(d, c):
            if c == 0:
                return x0d[d][:]
            if c == 1:
                return x1d[d][:]
            o = (c - 2) * _CHUNK
            return xrd[d][:, o : o + _CHUNK]

        def w1v(d, hh):
            return w1d[d][:, hh * 128 : (hh + 1) * 128]

        def w2v(hh, d2):
            return w2d[hh][:, d2 * 128 : (d2 + 1) * 128]

        # ---- main loop: one iteration computes gating+mm1 of chunk n while
        # running mm2+drain of chunk c, interleaved per h-tile on the PE so
        # gating's ACT/DVE round-trips hide behind matmul work ---------------
        wb_tiles = {}
        e_by_chunk = {}
        ht_by_chunk = {}
        macc = const.tile([1, 1], F32)

        def gating_head(n):
            # logits -> exp; PE + ACT only, no downstream waits
            psg = pgs.tile([_E, _CHUNK], F32, tag="pg")
            for d in range(_DT):
                nc.tensor.matmul(
                    psg[:], wgx_sb[:, d * _E : (d + 1) * _E], xm(d, n),
                    start=(d == 0), stop=(d == _DT - 1),
                )
            e_sb = epool.tile([_E, _CHUNK], F32R, tag="e_sb")
            nc.scalar.activation(e_sb[:], psg[:], AF.Exp, bias=gb_sb[:])
            e_by_chunk[n] = e_sb

        def gating_sum(n):
            # S = column sum over experts; own expert's row is partition 0
            e_sb = e_by_chunk[n]
            pss = pgs.tile([1, _CHUNK], F32, tag="pg")
            nc.tensor.matmul(
                pss[:], onesm[:, 0:1], e_sb[:], start=True, stop=True
            )
            recip = small.tile([1, _CHUNK], F32, tag="recip")
            nc.vector.reciprocal_approx_fast(recip[:], pss[0:1, :])
            wu = wrp.tile([1, _CHUNK], F32R, tag="wu")
            nc.vector.tensor_tensor(
                wu[:], e_sb[0:1, :].bitcast(F32), recip[:], ALU.mult
            )
            return wu

        def gating_wb(n, wu):
            # wb[128, CHUNK] = ones[1,128].T @ wu — PE outer-product bcast
            wbps = pgs.tile([128, _CHUNK], F32, tag="pg")
            nc.tensor.matmul(
                wbps[:], onesm[0:1, :], wu[:], start=True, stop=True
            )
            wb = wbp.tile([128, _CHUNK], F32, tag="wb")
            nc.vector.tensor_copy(wb[:], wbps[:])
            wb_tiles[n] = wb
            del e_by_chunk[n]
            # active criterion: max_b w > thresh, max-accumulated into macc
            rmax = crp.tile([1, 1], F32, tag="rmax", name=f"rmax_{n}")
            nc.vector.reduce_max(rmax[:], wu[:].bitcast(F32), axis=AX.X)
            if n == 0:
                nc.vector.tensor_copy(macc[:], rmax[:])
            else:
                nc.vector.tensor_tensor(macc[:], macc[:], rmax[:], ALU.max)

        def mm1_tile(n, hh):
            psh = ph.tile([128, _CHUNK], F32, tag="psh")
            for d in range(_DT):
                nc.tensor.matmul(
                    psh[:], w1v(d, hh), xm(d, n),
                    start=(d == 0), stop=(d == _DT - 1),
                )
            ht = htp.tile([128, _CHUNK], MMDT, tag="ht")
            nc.scalar.activation(ht[:], psh[:], AF.Tanh, bias=b1_sb[:, hh : hh + 1])
            ht_by_chunk.setdefault(n, []).append(ht)

        def iteration(n, c):
            # n: chunk for gating+mm1 (None to skip); c: chunk for mm2+drain
            pso_tiles = None
            ht_prev = None
            if c is not None:
                pso_tiles = [
                    po.tile([128, _CHUNK], F32, tag="pso", name=f"pso_{c}_{d2}")
                    for d2 in range(_DT)
                ]
                ht_prev = ht_by_chunk.pop(c)
            if n is not None:
                gating_head(n)
                wu = None
            for hh in range(_HT):
                if n is not None:
                    mm1_tile(n, hh)
                if c is not None:
                    for d2 in range(_DT):
                        nc.tensor.matmul(
                            pso_tiles[d2][:],
                            w2v(hh, d2),
                            ht_prev[hh][:],
                            start=(hh == 0), stop=(hh == _HT - 1),
                        )
                if n is not None and hh == 0:
                    wu = gating_sum(n)
                if n is not None and hh == 2:
                    gating_wb(n, wu)
            if c is not None:
                cs = slice(c * _CHUNK, (c + 1) * _CHUNK)
                wb = wb_tiles.pop(c)
                for d2 in range(_DT):
                    osb = op.tile([128, _CHUNK], MMIO, tag="osb")
                    nc.vector.tensor_tensor(
                        osb[:], pso_tiles[d2][:], wb[:], ALU.mult
                    )
                    nc.sync.dma_start(
                        OUTT.ap()[d2 * 128 : (d2 + 1) * 128, cs], osb[:]
                    )

        iteration(0, None)
        for c in range(_NCH):
            n = c + 1
            iteration(n if n < _NCH else None, c)

        # active mask -> MACT [1,1] (exact 0.0/1.0), applied host-side
        mact = crp.tile([1, 1], F32, tag="mact")
        nc.vector.tensor_scalar(mact[:], macc[:], _THRESH, None, ALU.is_gt)
        nc.sync.dma_start(MACT.ap(), mact[:])

    nc.finalize()
    return nc


def _get_nc():
    key = ("nc", _MM_BF16)
    if key not in _CACHE:
        _CACHE[key] = _build(_MM_BF16)
    return _CACHE[key]


def _make_in_maps(t, x, W1, b1, W2, b2, Wg, bg):
    import ml_dtypes

    mmdt = ml_dtypes.bfloat16 if _MM_BF16 else np.float32
    xTm = np.ascontiguousarray(x.T.astype(mmdt))
    wgx = Wg[:_D].astype(mmdt)
    gb = (np.float32(t[0]) * Wg[2 * _D] + bg).astype(np.float32).reshape(_E, 1)
    onesm = np.ones((_E, 128), dtype=np.float32)
    in_maps = []
    for c in range(_NCORES):
        # own expert first: E_e lands on partition 0 of the gating layout
        perm = [c] + [e for e in range(_E) if e != c]
        in_maps.append(
            {
                "XT": xTm,
                "W1E": np.ascontiguousarray(W1[c].astype(mmdt)),
                "W2E": np.ascontiguousarray(W2[c].astype(mmdt)),
                "B1E": np.ascontiguousarray(
                    b1[c].reshape(_HT, 128).T, dtype=np.float32
                ),
                "WGX": np.ascontiguousarray(wgx[:, perm]),
                "GB": np.ascontiguousarray(gb[perm]),
                "ONESM": onesm,
            }
        )
    return in_maps


def _assemble(results, inputs):
    out = np.zeros((_B, _D), dtype=np.float64)
    for c in range(_NCORES):
        if results[c]["MACT"][0, 0] > 0.5:
            out += results[c]["OUTT"].astype(np.float64).T
    b2 = np.asarray(inputs["b2"])
    if np.any(b2):
        # rank-1 bias term sum_e m_e * w[:,e] b2[e,:] — numpy gating replay
        t, x, Wg, bg = (np.asarray(inputs[k]) for k in ("t", "x", "Wg", "bg"))
        logits = x.astype(np.float64) @ Wg[:_D].astype(np.float64)
        logits += np.float64(t[0]) * Wg[2 * _D].astype(np.float64) + bg
        ex = np.exp(logits - logits.max(axis=1, keepdims=True))
        w = ex / ex.sum(axis=1, keepdims=True)
        active = (w > _THRESH).any(axis=0)
        out += (w * active) @ b2.astype(np.float64)
    return out.astype(np.float32)


def run_on_device(t, x, W1, b1, W2, b2, Wg, bg, trace=False):
    from concourse.bass_utils import run_bass_kernel_spmd

    inputs = dict(t=t, x=x, W1=W1, b1=b1, W2=W2, b2=b2, Wg=Wg, bg=bg)
    in_maps = _make_in_maps(**inputs)
    res = run_bass_kernel_spmd(
        _get_nc(), in_maps, list(range(_NCORES)), trace=trace
    )
    return _assemble(res.results, inputs), res


def kernel(t, x, W1, b1, W2, b2, Wg, bg):
    out, _ = run_on_device(t, x, W1, b1, W2, b2, Wg, bg, trace=False)
    return out
